# revision 7
# baseline (speedup 1.0000x reference)
"""Trainium2 Bass kernel for nn_ExpertChoiceRouter (moe_routing).

Contract: kernel(**inputs) takes FULL inputs
  hidden_states [4, 4096, 2048] f32, theta [3, 2048] f32
and returns the FULL output tuple (depth [4,4096] i32, balancing_loss f32,
masks [3,4,4096] bool) matching reference().

Strategy:
  - The memory-bound work is reading hidden_states (134 MB) and computing
    logits = hidden @ theta.T  -> [16384, 3].  That runs on 8 NeuronCores,
    data-parallel over tokens (2048 tokens/core).
  - Per core the Bass/Tile program loads X tiles [128 tok, 2048 h],
    transposes 128x128 blocks on the PE (fp32), copies PSUM->SBUF
    (alternating DVE/ACT), then accumulates theta-stationary fp32 matmuls
    out[3, 128 tok] += thetaT_c.T @ Xt_c over 16 h-chunks.
  - The tiny top-k / loss epilogue runs on host with the exact same jnp ops
    (same default backend) as the reference, fed by the device logits.
    This matters: the backend's sigmoid saturates to 1.0 at x~16.636 and
    ~35% of logits saturate, so the top-k is decided by ties at 1.0 broken
    by index; replaying the reference ops on the same backend reproduces
    the selection exactly (nearest logit to the cutoff is ~1.8e-3 away,
    while fp32 matmul error is ~1e-4).
"""

import numpy as np

_B, _S, _H, _D = 4, 4096, 2048, 3
_N_CORES = 8
_TOK = _B * _S            # 16384 tokens
_TPC = _TOK // _N_CORES   # 2048 tokens per core
_HC = _H // 128           # 16 h-chunks
_TB = _TPC // 128         # 16 token blocks per core

_prog_cache = {}


def _build_program():
    from contextlib import ExitStack

    import concourse.bass as bass
    import concourse.tile as tile
    from concourse import bacc, masks, mybir

    nc = bacc.Bacc(
        "TRN2", target_bir_lowering=False, debug=False, enable_asserts=False
    )
    x = nc.dram_tensor("x", [_TPC, _H], mybir.dt.float32, kind="ExternalInput").ap()
    theta = nc.dram_tensor(
        "theta", [_D, _H], mybir.dt.float32, kind="ExternalInput"
    ).ap()
    out = nc.dram_tensor(
        "logits", [_D, _TPC], mybir.dt.float32, kind="ExternalOutput"
    ).ap()

    with tile.TileContext(nc) as tc, ExitStack() as ctx:
        const_pool = ctx.enter_context(tc.tile_pool(name="const", bufs=1))
        x_pool = ctx.enter_context(tc.tile_pool(name="xin", bufs=3))
        xt_pool = ctx.enter_context(tc.tile_pool(name="xt", bufs=2 * _HC + 2))
        ps_tr_pool = ctx.enter_context(tc.tile_pool(name="pstr", bufs=4, space="PSUM"))
        ps_out_pool = ctx.enter_context(tc.tile_pool(name="psout", bufs=2, space="PSUM"))
        res_pool = ctx.enter_context(tc.tile_pool(name="res", bufs=1))

        ident = const_pool.tile([128, 128], mybir.dt.float32)
        masks.make_identity(nc, ident[:])
        # Dummy transpose so the PE observes the identity's producer (gpsimd)
        # here; walrus allows only ONE sync-wait per Matmult, and the first
        # real transpose must spend its one wait on the x-tile DMA.
        ps_warm = ps_tr_pool.tile([128, 128], mybir.dt.float32, tag="ps_tr")
        nc.tensor.transpose(ps_warm[:], ident[:], ident[:])

        # thetaT[p, c*3+d] = theta[d, c*128+p]
        thetaT = const_pool.tile([128, _HC * _D], mybir.dt.float32)
        for c in range(_HC):
            nc.sync.dma_start(
                out=thetaT[:, c * _D : (c + 1) * _D],
                in_=theta[:, c * 128 : (c + 1) * 128].rearrange("d p -> p d"),
            )

        logits_sb = res_pool.tile([_D, _TPC], mybir.dt.float32)

        for t in range(_TB):
            xt = x_pool.tile([128, _H], mybir.dt.float32)
            nc.sync.dma_start(out=xt[:], in_=x[t * 128 : (t + 1) * 128, :])

            xts = []
            for c in range(_HC):
                ps_tr = ps_tr_pool.tile([128, 128], mybir.dt.float32)
                nc.tensor.transpose(
                    ps_tr[:], xt[:, c * 128 : (c + 1) * 128], ident[:]
                )
                xt_sb = xt_pool.tile([128, 128], mybir.dt.float32)
                if c % 2 == 0:
                    nc.vector.tensor_copy(xt_sb[:], ps_tr[:])
                else:
                    nc.scalar.copy(xt_sb[:], ps_tr[:])
                xts.append(xt_sb)

            ps_out = ps_out_pool.tile([_D, 128], mybir.dt.float32)
            for c in range(_HC):
                nc.tensor.matmul(
                    ps_out[:],
                    thetaT[:, c * _D : (c + 1) * _D],
                    xts[c][:],
                    start=(c == 0),
                    stop=(c == _HC - 1),
                )
            nc.scalar.copy(logits_sb[:, t * 128 : (t + 1) * 128], ps_out[:])

        nc.sync.dma_start(out=out[:], in_=logits_sb[:])
    nc.compile()
    return nc


def _run_device_logits(hidden_states: np.ndarray, theta: np.ndarray):
    """Run the SPMD Bass kernel on 8 cores; return logits [B, S, D] f32."""
    from concourse import bass_utils

    if "nc" not in _prog_cache:
        _prog_cache["nc"] = _build_program()
    nc = _prog_cache["nc"]

    xflat = np.ascontiguousarray(
        hidden_states.reshape(_TOK, _H).astype(np.float32, copy=False)
    )
    theta = np.ascontiguousarray(theta.astype(np.float32, copy=False))
    in_maps = [
        {"x": xflat[i * _TPC : (i + 1) * _TPC], "theta": theta}
        for i in range(_N_CORES)
    ]
    res = bass_utils.run_bass_kernel_spmd(
        nc, in_maps, list(range(_N_CORES)), **_prog_cache.get("run_kwargs", {})
    )
    _prog_cache["last_results"] = res
    logitsT = np.concatenate(
        [np.asarray(res.results[i]["logits"]) for i in range(_N_CORES)], axis=1
    )  # [D, TOK]
    return logitsT.T.reshape(_B, _S, _D)


def _postprocess(logits_bsd: np.ndarray):
    """Exactly the reference's post-einsum code, on the same jnp backend."""
    import jax
    import jax.numpy as jnp

    B, S, D = _B, _S, _D
    k = max(1, int(S * (1.0 / D)))

    scores = jax.nn.sigmoid(jnp.asarray(logits_bsd))

    depth = jnp.zeros((B, S), dtype=jnp.int32)
    active = jnp.ones((B, S), dtype=bool)
    bidx = jnp.arange(B)[:, None]
    neg_inf = jnp.float32(-jnp.inf)

    for r in range(D):
        masked = jnp.where(active, scores[:, :, r], neg_inf)
        _, idx = jax.lax.top_k(masked, k)
        depth = depth.at[bidx, idx].set(r + 1)
        step_mask = jnp.zeros((B, S), dtype=bool).at[bidx, idx].set(True)
        active = active & step_mask

    depth = jnp.where(depth == 0, 1, depth)

    probs = jax.nn.sigmoid(scores).mean(axis=(0, 1))
    target = jnp.float32(1.0 / D)
    balancing_loss = jnp.sum(target * (jnp.log(target) - jnp.log(probs))) / D

    masks = jnp.stack(
        [jnp.ones((B, S), dtype=bool) if r == 0 else active for r in range(D)]
    )
    return (
        np.asarray(depth),
        np.asarray(balancing_loss),
        np.asarray(masks),
    )


def kernel(hidden_states: np.ndarray, theta: np.ndarray):
    logits = _run_device_logits(hidden_states, theta)
    return _postprocess(logits)


if __name__ == "__main__":
    rng = np.random.default_rng(0)
    hs = rng.standard_normal((_B, _S, _H), dtype=np.float32)
    th = rng.standard_normal((_D, _H), dtype=np.float32)
    out = kernel(hidden_states=hs, theta=th)
    print([o.shape for o in out])


# revision 22
# speedup vs baseline: 1.3082x; 1.3082x over previous
"""Trainium2 Bass kernel for nn_ExpertChoiceRouter (moe_routing).

kernel(**inputs) takes FULL inputs
  hidden_states [4, 4096, 2048] f32, theta [3, 2048] f32
and returns the FULL output tuple (depth [4,4096] i32, balancing_loss f32,
masks [3,4,4096] bool) matching the reference nn.Module.

Device side (8 NeuronCores, data-parallel over the 16384 tokens, 2048/core):
compute logits = X @ theta.T in true fp32.  Per core the 16 [128tok, 2048h]
tiles are split across two compute paths so every engine runs near the
~47us/core HBM roofline:

  - PE path (tiles 0..PE_TILES-1, groups of GRP): PE transposes 128x128
    blocks (fp32) into [128, 128*GRP] PSUM groups, DVE/ACT copy PSUM->SBUF,
    then fp32 matmuls (theta stationary, wide moving operand) accumulate
    logitsT [3, tok] over the 16 h-chunks.  Cheap bf16 dummy matmuls are
    interleaved as a heartbeat so the PE HAM clock-gate stays at 2.4 GHz
    (transpose-mode ops don't count as PE activity and the PE otherwise
    decays to 1.2 GHz).

  - Elementwise path (remaining tiles): DVE tensor_tensor multiplies the
    natural [128tok, 2048h] tile by theta_d replicated across partitions;
    ACT's ACTIVATE(Copy, accum_out=..) reduces the product along the free
    dim -> logits [128tok, 1] per d.  No transpose needed.  GPSIMD is
    deliberately NOT used for elementwise work: it shares SBUF ports with
    the DVE and concurrent streaming makes both slower than DVE alone.

Host side: the tiny top-k / loss epilogue replays the reference's exact
jnp ops on the same default backend, fed with the device logits.  This is
required for exactness: the backend's sigmoid saturates to 1.0 at x~16.636
and ~35% of logits saturate, so the top-k is decided by ties at 1.0 broken
by lowest index.  The nearest logit to the saturation cutoff is ~1.8e-3
away while the device matmul error is ~2e-4, so exact-fp32 logits +
same-backend sigmoid/top_k reproduce the reference selection bit-for-bit.
"""

import numpy as np

_B, _S, _H, _D = 4, 4096, 2048, 3
_N_CORES = 8
_TOK = _B * _S            # 16384 tokens
_TPC = _TOK // _N_CORES   # 2048 tokens per core
_HC = _H // 128           # 16 h-chunks
_TB = _TPC // 128         # 16 token blocks per core

# --- schedule knobs (per core) ---
PE_TILES = 8              # tiles on the PE path
GRP = 4                   # max PE-path tiles per wide matmul group
GROUPS = [4, 4]           # PE-path group sizes (sum == PE_TILES)
MM_LAG = 2                # matmul trails its transposes by this many chunks
WARMUP_MMS = 20           # dummy matmuls at start to engage the HAM clock gate
HEARTBEAT = True          # bf16 dummy mm after each chunk's transposes
# of the GRP*? copies per group, which chunks go to DVE (rest ACT)
COPY_DVE_CHUNKS = 5       # chunks c < this -> DVE copy, else ACT
PE_RATIO = 1.2            # pe chunk-steps emitted per dve unit
RED_DVE_TAIL = 1          # last N dve tiles reduce on DVE instead of ACT
# emission order: elementwise tiles first so DVE/ACT start early
EMIT_ORDER = (
    [("dve", 0), ("dve", 1), ("dve", 2)]
    + [("pe", 0)]
    + [("dve", 3), ("dve", 4), ("dve", 5)]
    + [("pe", 1)]
    + [("dve", 6), ("dve", 7)]
)

_prog_cache = {}


def _build_program():
    from contextlib import ExitStack

    import concourse.tile as tile
    from concourse import bacc, masks, mybir

    f32 = mybir.dt.float32
    bf16 = mybir.dt.bfloat16
    DVE_TILES = _TB - PE_TILES
    g_base = [sum(GROUPS[:i]) for i in range(len(GROUPS))]

    nc = bacc.Bacc(
        "TRN2", target_bir_lowering=False, debug=False, enable_asserts=False
    )
    x = nc.dram_tensor("x", [_TPC, _H], f32, kind="ExternalInput").ap()
    # host-precomputed: theta_rep[p, d*H+h] = theta[d, h] (tiled over 128
    # partitions); thetaT_in[p, c*D+d] = theta[d, c*128+p]
    theta_rep_in = nc.dram_tensor(
        "theta_rep", [128, _D * _H], f32, kind="ExternalInput"
    ).ap()
    thetaT_in = nc.dram_tensor(
        "thetaT", [128, _HC * _D], f32, kind="ExternalInput"
    ).ap()
    out_pe = nc.dram_tensor(
        "logits_pe", [_D, PE_TILES * 128], f32, kind="ExternalOutput"
    ).ap()
    out_dve = nc.dram_tensor(
        "logits_dve", [DVE_TILES * 128, _D], f32, kind="ExternalOutput"
    ).ap()

    with tile.TileContext(nc) as tc, ExitStack() as ctx:
        const_pool = ctx.enter_context(tc.tile_pool(name="const", bufs=1))
        x_pool = ctx.enter_context(tc.tile_pool(name="xin", bufs=10))
        wide_pool = ctx.enter_context(tc.tile_pool(name="wide", bufs=17))
        prod_pool = ctx.enter_context(tc.tile_pool(name="prod", bufs=4))
        acc_pool = ctx.enter_context(tc.tile_pool(name="acc", bufs=3))
        ps_tr_pool = ctx.enter_context(tc.tile_pool(name="pstr", bufs=4, space="PSUM"))
        ps_out_pool = ctx.enter_context(tc.tile_pool(name="psout", bufs=2, space="PSUM"))
        ps_hb_pool = ctx.enter_context(tc.tile_pool(name="pshb", bufs=1, space="PSUM"))
        res_pool = ctx.enter_context(tc.tile_pool(name="res", bufs=1))

        theta_rep = const_pool.tile([128, _D * _H], f32)
        thetaT = const_pool.tile([128, _HC * _D], f32)

        ident = const_pool.tile([128, 128], f32)
        masks.make_identity(nc, ident[:])
        identb = const_pool.tile([128, 128], bf16)
        masks.make_identity(nc, identb[:])

        # Absorb the identities' gpsimd-producer wait: walrus allows only ONE
        # sync-wait per Matmult, and the first real transpose must spend its
        # wait on an x-tile DMA.
        ps_hb = ps_hb_pool.tile([128, 128], f32)
        nc.tensor.transpose(ps_hb[:], ident[:], ident[:])
        # Dummy bf16 matmul burst engages the HAM clock gate while DMAs land.
        for _ in range(WARMUP_MMS):
            nc.tensor.matmul(
                ps_hb[:], identb[:], identb[:], start=True, stop=True
            )


        logits_sb = res_pool.tile([_D, PE_TILES * 128], f32)
        red_scratch = res_pool.tile([128, _H], bf16)

        xtiles = {}

        def load_x(t, eng):
            xt = x_pool.tile([128, _H], f32, tag="xt_in")
            eng.dma_start(out=xt[:], in_=x[t * 128 : (t + 1) * 128, :])
            xtiles[t] = xt

        # Explicit DMA prologue: interleave the two HWDGE rings so the first
        # work unit of each path is fed as early as possible, and prefetch
        # the PE groups' tiles so the PE never starves mid-kernel.
        load_x(PE_TILES, nc.sync)                   # first elementwise tile
        nc.scalar.dma_start(
            out=theta_rep[:, 0:_H], in_=theta_rep_in[:, 0:_H]
        )
        pe_todo = list(range(PE_TILES))
        extra = [
            lambda: nc.scalar.dma_start(
                out=theta_rep[:, _H : 2 * _H],
                in_=theta_rep_in[:, _H : 2 * _H],
            ),
            lambda: load_x(PE_TILES + 1, nc.sync),
            lambda: nc.scalar.dma_start(
                out=theta_rep[:, 2 * _H :], in_=theta_rep_in[:, 2 * _H :]
            ),
            lambda: nc.scalar.dma_start(out=thetaT[:], in_=thetaT_in[:]),
        ]
        k = 0
        while pe_todo:
            t = pe_todo.pop(0)
            load_x(t, nc.sync if t % 2 == 1 else nc.scalar)
            if t % 2 == 1 and k < len(extra):
                extra[k]()
                k += 1
        while k < len(extra):
            extra[k]()
            k += 1

        def pe_group_steps(g):
            gsz = GROUPS[g]
            nw = 128 * gsz
            xts = [xtiles[g_base[g] + tt] for tt in range(gsz)]
            ps_out = ps_out_pool.tile([_D, nw], f32, tag="ps_out")
            wides = []
            # Phase A: all transposes + copies.  Phase B: a dense burst of
            # the 16 accumulating matmuls.  Transpose-mode ops don't count
            # as PE activity for the HAM clock gate, so mixing them with the
            # matmuls keeps the PE at 1.2 GHz; a dense matmul burst warms to
            # 2.4 GHz after ~3.4us and nearly halves the matmul time.
            for c in range(_HC):
                ps_tr = ps_tr_pool.tile([128, nw], f32, tag="ps_tr")
                for tt in range(gsz):
                    nc.tensor.transpose(
                        ps_tr[:, tt * 128 : (tt + 1) * 128],
                        xts[tt][:, c * 128 : (c + 1) * 128],
                        ident[:],
                    )
                wide = wide_pool.tile([128, nw], f32, tag="wide")
                if c < COPY_DVE_CHUNKS:
                    nc.vector.tensor_copy(wide[:], ps_tr[:])
                else:
                    nc.scalar.copy(wide[:], ps_tr[:])
                wides.append(wide)
                yield
            for cc in range(_HC):
                nc.tensor.matmul(
                    ps_out[:],
                    thetaT[:, cc * _D : (cc + 1) * _D],
                    wides[cc][:],
                    start=(cc == 0),
                    stop=(cc == _HC - 1),
                )
                if cc % 4 == 3:
                    yield
            lo = g_base[g] * 128
            hi = lo + nw
            nc.scalar.copy(logits_sb[:, lo:hi], ps_out[:])
            nc.sync.dma_start(out=out_pe[:, lo:hi], in_=logits_sb[:, lo:hi])
            yield

        def dve_tile_units(i):
            t = PE_TILES + i
            if t not in xtiles:
                load_x(t, nc.sync)
            xt = xtiles[t]
            acc = acc_pool.tile([128, _D], f32)
            for d in range(_D):
                prod = prod_pool.tile([128, _H], f32, tag="prod")
                nc.vector.tensor_tensor(
                    out=prod[:],
                    in0=xt[:],
                    in1=theta_rep[:, d * _H : (d + 1) * _H],
                    op=mybir.AluOpType.mult,
                )
                if i >= DVE_TILES - RED_DVE_TAIL:
                    nc.vector.reduce_sum(
                        acc[:, d : d + 1], prod[:], axis=mybir.AxisListType.X
                    )
                else:
                    nc.scalar.activation(
                        out=red_scratch[:],
                        in_=prod[:],
                        func=mybir.ActivationFunctionType.Copy,
                        accum_out=acc[:, d : d + 1],
                    )
                yield
            nc.sync.dma_start(
                out=out_dve[i * 128 : (i + 1) * 128, :], in_=acc[:]
            )
            yield

        NG = len(GROUPS)
        pe_gens = [pe_group_steps(g) for g in range(NG)]
        dve_gens = [dve_tile_units(i) for i in range(DVE_TILES)]
        credit = 0.0
        gi, di2 = 0, 0
        while gi < NG or di2 < DVE_TILES:
            advanced = False
            if di2 < DVE_TILES:
                try:
                    next(dve_gens[di2])
                    advanced = True
                except StopIteration:
                    di2 += 1
                    continue
            credit += PE_RATIO
            while credit >= 1.0 and gi < NG:
                try:
                    next(pe_gens[gi])
                    credit -= 1.0
                    advanced = True
                except StopIteration:
                    gi += 1
            if di2 >= DVE_TILES and gi < NG:
                try:
                    next(pe_gens[gi])
                except StopIteration:
                    gi += 1
            if not advanced and di2 >= DVE_TILES and gi >= NG:
                break
    nc.compile()
    return nc


def _run_device_logits(hidden_states: np.ndarray, theta: np.ndarray):
    """Run the SPMD Bass kernel on 8 cores; return logits [B, S, D] f32."""
    from concourse import bass_utils

    if "nc" not in _prog_cache:
        _prog_cache["nc"] = _build_program()
    nc = _prog_cache["nc"]

    xflat = np.ascontiguousarray(
        hidden_states.reshape(_TOK, _H).astype(np.float32, copy=False)
    )
    theta = np.ascontiguousarray(theta.astype(np.float32, copy=False))
    theta_rep = np.tile(theta.reshape(1, _D * _H), (128, 1))
    thetaT = np.ascontiguousarray(
        theta.reshape(_D, _HC, 128).transpose(2, 1, 0).reshape(128, _HC * _D)
    )
    in_maps = [
        {
            "x": xflat[i * _TPC : (i + 1) * _TPC],
            "theta_rep": theta_rep,
            "thetaT": thetaT,
        }
        for i in range(_N_CORES)
    ]
    res = bass_utils.run_bass_kernel_spmd(
        nc, in_maps, list(range(_N_CORES)), **_prog_cache.get("run_kwargs", {})
    )
    _prog_cache["last_results"] = res

    logits = np.empty((_TOK, _D), dtype=np.float32)
    for i in range(_N_CORES):
        base = i * _TPC
        lp = np.asarray(res.results[i]["logits_pe"])   # [D, PE_TILES*128]
        ld = np.asarray(res.results[i]["logits_dve"])  # [DVE_TILES*128, D]
        logits[base : base + PE_TILES * 128] = lp.T
        logits[base + PE_TILES * 128 : base + _TPC] = ld
    return logits.reshape(_B, _S, _D)


def _postprocess(logits_bsd: np.ndarray):
    """Exactly the reference's post-einsum code, on the same jnp backend."""
    import jax
    import jax.numpy as jnp

    B, S, D = _B, _S, _D
    k = max(1, int(S * (1.0 / D)))

    scores = jax.nn.sigmoid(jnp.asarray(logits_bsd))

    depth = jnp.zeros((B, S), dtype=jnp.int32)
    active = jnp.ones((B, S), dtype=bool)
    bidx = jnp.arange(B)[:, None]
    neg_inf = jnp.float32(-jnp.inf)

    for r in range(D):
        masked = jnp.where(active, scores[:, :, r], neg_inf)
        _, idx = jax.lax.top_k(masked, k)
        depth = depth.at[bidx, idx].set(r + 1)
        step_mask = jnp.zeros((B, S), dtype=bool).at[bidx, idx].set(True)
        active = active & step_mask

    depth = jnp.where(depth == 0, 1, depth)

    probs = jax.nn.sigmoid(scores).mean(axis=(0, 1))
    target = jnp.float32(1.0 / D)
    balancing_loss = jnp.sum(target * (jnp.log(target) - jnp.log(probs))) / D

    masks = jnp.stack(
        [jnp.ones((B, S), dtype=bool) if r == 0 else active for r in range(D)]
    )
    return (
        np.asarray(depth),
        np.asarray(balancing_loss),
        np.asarray(masks),
    )


def kernel(hidden_states: np.ndarray, theta: np.ndarray):
    logits = _run_device_logits(hidden_states, theta)
    return _postprocess(logits)


if __name__ == "__main__":
    rng = np.random.default_rng(0)
    hs = rng.standard_normal((_B, _S, _H), dtype=np.float32)
    th = rng.standard_normal((_D, _H), dtype=np.float32)
    out = kernel(hidden_states=hs, theta=th)
    print([o.shape for o in out])
